# revision 53
# baseline (speedup 1.0000x reference)
"""Bahdanau (additive) attention Trainium2 Bass kernel.

Reference math (per batch b):
    wq  = query @ Wq + bq                      # [Lq, H]
    uh  = key @ Wk                             # [Lk, H]
    attn[q,k]   = sum_h v[h] * tanh(wq[q,h] + uh[k,h])      # [Lq, Lk]
    norm_attn   = softmax(attn, axis=-1)
    h           = norm_attn @ value            # [Lq, Dv]
    returns (h, attn, norm_attn)

Shapes: B=4, Lq=Lk=256, Dq=Dk=Dv=H=512, fp32.

Sharding: 8 cores = (batch b in 0..3) x (Lq half in 0..1); each core owns a
[128, :] slice of Lq for one batch. No collectives.

Per-core layout strategy ("transposed"): keep H on partitions.
  W[h_chunk partitions, (hc, q) free]  = (query @ Wq + bq)^T   (4 chunks of 128)
  U[h_chunk partitions, (hc, k) free]  = (key @ Wk)^T
  For each q: X[:, k] = U + W[:, q] broadcast (DVE/Pool tensor_scalar add,
  per-partition scalar), T = tanh(X) (ScalarE, one big fp32r instruction per
  q-group; ScalarE is the only tanh engine and the kernel's roofline at
  ~109us/core), then attn is accumulated on the TensorEngine: the stationary
  operand is a sliding [128, 64] window over a zero buffer with v at one
  column, so matmul q deposits sum_h v_h * T[h, k] into PSUM row q%64; the
  512 accumulating matmuls assemble attn [q, k] in two half banks in place.
Per 64-row half: softmax along the free dim, then h = norm_attn @ value via
PE transpose, pipelined behind the second half's main loop.  All matmul
inputs use float32r (4x PE throughput, ~1e-4 output error).
"""

import os
import numpy as np

B, LQ, LK, D, H, DV = 4, 256, 256, 512, 512, 512
P = 128  # partitions
NQ = LQ // 2          # q rows per core
NHC = H // P          # 4 h-chunks
NDC = D // P          # 4 d-chunks
NKC = LK // P         # 2 k-chunks
HQ = 64               # q's per attn psum half (M of the matvec matmuls)
# ramp group sizes: small first groups start ScalarE early, small last ones
# shrink the kernel tail
GROUP_SIZES = [1, 1, 2, 4] + [7] * 16 + [4, 2, 1, 1]
assert sum(GROUP_SIZES) == NQ
GMAX = max(GROUP_SIZES)

_NC_CACHE = {}


def _build_nc(use_preadd: bool = True):
    import concourse.tile as tile
    from concourse import bacc, mybir
    from concourse.masks import make_identity

    fp32 = mybir.dt.float32
    fp32r = mybir.dt.float32r
    AF = mybir.ActivationFunctionType

    nc = bacc.Bacc("TRN2", target_bir_lowering=False)

    qT_d = nc.dram_tensor("qT", [D, P], fp32r, kind="ExternalInput")
    kT_d = nc.dram_tensor("keyT", [D, LK], fp32r, kind="ExternalInput")
    val_d = nc.dram_tensor("val", [LK, DV], fp32r, kind="ExternalInput")
    Wq_d = nc.dram_tensor("Wq", [D, H], fp32r, kind="ExternalInput")
    Wk_d = nc.dram_tensor("Wk", [D, H], fp32r, kind="ExternalInput")
    bq_d = nc.dram_tensor("bq4", [P, NHC], fp32, kind="ExternalInput")
    zw_d = nc.dram_tensor("zwin", [P, NHC, 2 * HQ - 1], fp32r, kind="ExternalInput")

    attn_d = nc.dram_tensor("attn_out", [NQ, LK], fp32, kind="ExternalOutput")
    norm_d = nc.dram_tensor("norm_out", [NQ, LK], fp32, kind="ExternalOutput")
    h_d = nc.dram_tensor("h_out", [NQ, DV], fp32, kind="ExternalOutput")

    from contextlib import ExitStack
    with tile.TileContext(nc) as tc:
        with (
            tc.tile_pool(name="statics", bufs=1) as statics,
            tc.tile_pool(name="psum_pre", bufs=1, space="PSUM") as psum_pre,
            tc.tile_pool(name="psum_attn", bufs=1, space="PSUM") as psum_attn,
            tc.tile_pool(name="psum_end", bufs=2, space="PSUM") as psum_end,
            tc.tile_pool(name="endpool", bufs=2) as endpool,
            ExitStack() as ctx_stack,
        ):
            # ---- static loads ----
            tpool = ctx_stack.enter_context(tc.tile_pool(name="tpool", bufs=3))
            xpool = ctx_stack.enter_context(tc.tile_pool(name="xpool", bufs=2))
            QT = [statics.tile([P, P], fp32r, name=f"qt{i}", tag=f"qt{i}") for i in range(NDC)]
            KT = [statics.tile([P, LK], fp32r, name=f"kt{i}", tag=f"kt{i}") for i in range(NDC)]
            WQ = [statics.tile([P, H], fp32r, name=f"wq{i}", tag=f"wq{i}") for i in range(NDC)]
            WK = [statics.tile([P, H], fp32r, name=f"wk{i}", tag=f"wk{i}") for i in range(NDC)]
            VAL = [statics.tile([P, DV], fp32r, name=f"val{i}", tag=f"val{i}") for i in range(NKC)]
            BQ = statics.tile([P, NHC], fp32)
            ZW = statics.tile([P, NHC, 2 * HQ - 1], fp32r)
            ident = statics.tile([P, P], fp32)

            # warm the ACT tanh/exp table set before any data arrives
            warm = statics.tile([P, 1], fp32)
            nc.vector.memset(warm, 0.0)
            nc.scalar.activation(warm, warm, AF.Tanh)

            nc.sync.dma_start(BQ, bq_d[:, :])
            nc.sync.dma_start(ZW, zw_d[:, :, :])
            for dc in range(NDC):
                nc.gpsimd.dma_start(KT[dc], kT_d[dc * P:(dc + 1) * P, :])
                nc.sync.dma_start(WK[dc], Wk_d[dc * P:(dc + 1) * P, :])
            for dc in range(NDC):
                nc.gpsimd.dma_start(QT[dc], qT_d[dc * P:(dc + 1) * P, :])
                nc.sync.dma_start(WQ[dc], Wq_d[dc * P:(dc + 1) * P, :])
            for kc in range(NKC):
                nc.gpsimd.dma_start(VAL[kc], val_d[kc * P:(kc + 1) * P, :])
            make_identity(nc, ident[:, :])

            # ---- precompute W (wq^T + bq) and U (uh^T), H on partitions ----
            Wt = statics.tile([P, NHC * P], fp32)    # [h_in_chunk, (hc, q)]
            Ut = statics.tile([P, NHC * LK], fp32)   # [h_in_chunk, (hc, k)]
            with tc.high_priority():
                for hc in range(NHC):
                    pu = psum_pre.tile([P, LK], fp32, tag="pu", name=f"pu{hc}")
                    for dc in range(NDC):
                        nc.tensor.matmul(
                            pu, WK[dc][:, hc * P:(hc + 1) * P], KT[dc],
                            start=(dc == 0), stop=(dc == NDC - 1),
                        )
                    nc.vector.tensor_copy(Ut[:, hc * LK:(hc + 1) * LK], pu)
                    pw = psum_pre.tile([P, P], fp32, tag="pw", name=f"pw{hc}")
                    for dc in range(NDC):
                        nc.tensor.matmul(
                            pw, WQ[dc][:, hc * P:(hc + 1) * P], QT[dc],
                            start=(dc == 0), stop=(dc == NDC - 1),
                        )
                    nc.vector.tensor_scalar_add(
                        Wt[:, hc * P:(hc + 1) * P], pw, BQ[:, hc:hc + 1]
                    )

            # ---- main loop: tanh cube + attn accumulation (two q-halves) ----
            attn_psh = [
                psum_attn.tile([P, LK], fp32, name=f"aps{j}", tag=f"aps{j}")
                for j in range(NQ // HQ)
            ]
            h_psh = [
                psum_end.tile([P, DV], fp32, name=f"hps{j}", tag=f"hps{j}", bufs=1)
                for j in range(NQ // HQ)
            ]
            attn_sbh = [statics.tile([HQ, LK], fp32, name=f"asb{j}", tag=f"asb{j}")
                        for j in range(NQ // HQ)]
            e_sbh = [statics.tile([HQ, LK], fp32, name=f"esb{j}", tag=f"esb{j}")
                     for j in range(NQ // HQ)]
            norm_sbh = [statics.tile([HQ, LK], fp32, name=f"nsb{j}", tag=f"nsb{j}")
                        for j in range(NQ // HQ)]
            h_sbh = [statics.tile([HQ, DV], fp32, name=f"hsb{j}", tag=f"hsb{j}")
                     for j in range(NQ // HQ)]
            stat = [statics.tile([HQ, 4], fp32, name=f"st{j}", tag=f"st{j}")
                    for j in range(NQ // HQ)]

            def epilogue(j):
                """softmax + h for q rows [HQ*j, HQ*(j+1)) of this core.

                Everything runs at partition base 0; only the output DMAs
                place the rows at their DRAM offsets."""
                rs = slice(HQ * j, HQ * (j + 1))
                asb, esb, nsb, hsb, st = (attn_sbh[j], e_sbh[j], norm_sbh[j],
                                          h_sbh[j], stat[j])
                # No max-subtraction: |attn| <= ||v||_1 ~ 20, exp is fp32-safe
                # and softmax is algebraically identical. exp reads PSUM
                # directly so the h path doesn't wait for the attn copy.
                nc.scalar.activation(esb, attn_psh[j][0:HQ, :], AF.Exp)
                nc.vector.tensor_copy(asb, attn_psh[j][0:HQ, :])
                nc.sync.dma_start(attn_d[rs, :], asb)
                nc.vector.reduce_sum(st[:, 2:3], esb, axis=mybir.AxisListType.X)
                nc.vector.reciprocal(st[:, 3:4], st[:, 2:3])
                nc.vector.tensor_scalar_mul(nsb, esb, st[:, 3:4])
                nc.sync.dma_start(norm_d[rs, :], nsb)
                # h = diag(1/sum) @ (e @ value): transpose e (not norm_attn)
                # so the matmuls don't wait on sum/recip; the normalization
                # rides on the final PSUM->SBUF copy for free.
                for kc in range(NKC):
                    pt = psum_end.tile([P, HQ], fp32, tag="pt")
                    nc.tensor.transpose(
                        pt, esb[:, kc * P:(kc + 1) * P], ident[0:HQ, 0:HQ]
                    )
                    nat = endpool.tile([P, HQ], fp32r, tag="nat")
                    nc.vector.tensor_copy(nat, pt)
                    nc.tensor.matmul(
                        h_psh[j][0:HQ, :], nat, VAL[kc],
                        start=(kc == 0), stop=(kc == NKC - 1),
                    )
                if j == NQ // HQ - 1:
                    # final half: ScalarE is idle at the tail
                    nc.scalar.mul(hsb, h_psh[j][0:HQ, :], st[:, 3:4])
                else:
                    nc.vector.tensor_scalar_mul(hsb, h_psh[j][0:HQ, :],
                                                st[:, 3:4])
                nc.sync.dma_start(h_d[rs, :], hsb)

            q0 = 0
            for g, gsz in enumerate(GROUP_SIZES):
                if g < 2 and gsz == 1:
                    # ramp-in: per-hc ACT-with-bias (no DVE preadd dep), so
                    # tanh for chunk hc starts as soon as Ut/Wt[hc] land
                    T = tpool.tile([P, GMAX * NHC * LK], fp32r, tag="t")
                    q = q0
                    for hc in range(NHC):
                        off = hc * LK
                        nc.scalar.activation(
                            T[:, off:off + LK],
                            Ut[:, hc * LK:(hc + 1) * LK],
                            AF.Tanh,
                            bias=Wt[:, hc * P + q:hc * P + q + 1],
                        )
                elif use_preadd:
                    X = xpool.tile([P, GMAX * NHC * LK], fp32, tag="x")
                    T = tpool.tile([P, GMAX * NHC * LK], fp32r, tag="t")
                    for ql in range(gsz):
                        q = q0 + ql
                        for hc in range(NHC):
                            off = (ql * NHC + hc) * LK
                            eng = nc.gpsimd if hc == NHC - 1 else nc.vector
                            eng.tensor_scalar_add(
                                X[:, off:off + LK],
                                Ut[:, hc * LK:(hc + 1) * LK],
                                Wt[:, hc * P + q:hc * P + q + 1],
                            )
                    fd = gsz * NHC * LK
                    nc.scalar.activation(T[:, :fd], X[:, :fd], AF.Tanh)
                else:
                    T = tpool.tile([P, GMAX * NHC * LK], fp32r, tag="t")
                    for ql in range(gsz):
                        q = q0 + ql
                        for hc in range(NHC):
                            off = (ql * NHC + hc) * LK
                            nc.scalar.activation(
                                T[:, off:off + LK],
                                Ut[:, hc * LK:(hc + 1) * LK],
                                AF.Tanh,
                                bias=Wt[:, hc * P + q:hc * P + q + 1],
                            )
                for ql in range(gsz):
                    q = q0 + ql
                    j, r = q // HQ, q % HQ
                    for hc in range(NHC):
                        off = (ql * NHC + hc) * LK
                        nc.tensor.matmul(
                            attn_psh[j][0:HQ, :],
                            ZW[:, hc, (HQ - 1) - r:(2 * HQ - 1) - r],
                            T[:, off:off + LK],
                            start=(r == 0 and hc == 0),
                            stop=(r == HQ - 1 and hc == NHC - 1),
                        )
                q0 += gsz
                if q0 % HQ == 0:
                    epilogue(q0 // HQ - 1)

    nc.compile()
    return nc


def _get_nc(use_preadd: bool = True):
    key = use_preadd
    if key not in _NC_CACHE:
        _NC_CACHE[key] = _build_nc(use_preadd)
    return _NC_CACHE[key]


def _make_in_maps(query, key, value, Wq, bq, Wk, v):
    zwin = np.zeros((P, NHC, 2 * HQ - 1), dtype=np.float32)
    v4 = np.ascontiguousarray(v.reshape(NHC, P).T)        # [128, 4]
    zwin[:, :, HQ - 1] = v4
    bq4 = np.ascontiguousarray(bq.reshape(NHC, P).T)      # [128, 4]
    Wq = np.ascontiguousarray(Wq, dtype=np.float32)
    Wk = np.ascontiguousarray(Wk, dtype=np.float32)
    in_maps = []
    for core in range(8):
        b, half = core // 2, core % 2
        in_maps.append({
            "qT": np.ascontiguousarray(query[b, half * NQ:(half + 1) * NQ, :].T),
            "keyT": np.ascontiguousarray(key[b].T),
            "val": np.ascontiguousarray(value[b]),
            "Wq": Wq,
            "Wk": Wk,
            "bq4": bq4,
            "zwin": zwin,
        })
    return in_maps


def kernel(query, key, value, Wq, bq, Wk, v, _trace=False, _use_preadd=True):
    from concourse.bass_utils import run_bass_kernel_spmd

    query = np.asarray(query, dtype=np.float32)
    key = np.asarray(key, dtype=np.float32)
    value = np.asarray(value, dtype=np.float32)
    Wq = np.asarray(Wq, dtype=np.float32)
    bq = np.asarray(bq, dtype=np.float32)
    Wk = np.asarray(Wk, dtype=np.float32)
    v = np.asarray(v, dtype=np.float32)

    nc = _get_nc(_use_preadd)
    in_maps = _make_in_maps(query, key, value, Wq, bq, Wk, v)
    res = run_bass_kernel_spmd(nc, in_maps, core_ids=list(range(8)), trace=_trace)

    h = np.empty((B, LQ, DV), dtype=np.float32)
    attn = np.empty((B, LQ, LK), dtype=np.float32)
    norm = np.empty((B, LQ, LK), dtype=np.float32)
    for core in range(8):
        b, half = core // 2, core % 2
        r = res.results[core]
        sl = slice(half * NQ, (half + 1) * NQ)
        h[b, sl, :] = r["h_out"]
        attn[b, sl, :] = r["attn_out"]
        norm[b, sl, :] = r["norm_out"]
    if _trace:
        kernel._last_results = res
    return (h, attn, norm)


# ---------------------------------------------------------------------------
# CoreSim self-test of a single core's program (no hardware needed).
def _selftest_sim(use_preadd=True):
    from concourse.bass_interp import CoreSim

    rng = np.random.default_rng(0)
    query = rng.standard_normal((B, LQ, D), dtype=np.float32)
    key = rng.standard_normal((B, LK, D), dtype=np.float32)
    value = rng.standard_normal((B, LK, DV), dtype=np.float32)
    Wq = (rng.standard_normal((D, H)) * 0.05).astype(np.float32)
    bq = (rng.standard_normal((H,)) * 0.05).astype(np.float32)
    Wk = (rng.standard_normal((D, H)) * 0.05).astype(np.float32)
    v = (rng.standard_normal((H,)) * 0.05).astype(np.float32)

    nc = _build_nc(use_preadd)
    in_maps = _make_in_maps(query, key, value, Wq, bq, Wk, v)
    core = 3  # b=1, half=1
    sim = CoreSim(nc)
    for name, arr in in_maps[core].items():
        sim.tensor(name)[:] = arr
    if nc.partition_id_tensor is not None:
        sim.tensor(nc.partition_id_tensor.name)[:] = np.array(
            [[core]], dtype=np.uint32)
    sim.simulate(check_with_hw=False)

    # numpy reference for that core
    b, half = core // 2, core % 2
    qs = query[b, half * NQ:(half + 1) * NQ, :]
    wq = qs @ Wq + bq
    uh = key[b] @ Wk
    x = np.tanh(wq[:, None, :] + uh[None, :, :])
    attn_ref = np.einsum("qkh,h->qk", x, v)
    m = attn_ref.max(-1, keepdims=True)
    e = np.exp(attn_ref - m)
    norm_ref = e / e.sum(-1, keepdims=True)
    h_ref = norm_ref @ value[b]

    for name, ref in [("attn_out", attn_ref), ("norm_out", norm_ref),
                      ("h_out", h_ref)]:
        got = np.asarray(sim.tensor(name))
        err = np.abs(got - ref).max()
        rel = err / max(np.abs(ref).max(), 1e-30)
        print(f"{name}: maxabs={err:.3e} rel={rel:.3e}")


if __name__ == "__main__":
    _selftest_sim(use_preadd=os.environ.get("PREADD", "1") == "1")


# revision 69
# speedup vs baseline: 1.0288x; 1.0288x over previous
"""Bahdanau (additive) attention Trainium2 Bass kernel.

Reference math (per batch b):
    wq  = query @ Wq + bq                      # [Lq, H]
    uh  = key @ Wk                             # [Lk, H]
    attn[q,k]   = sum_h v[h] * tanh(wq[q,h] + uh[k,h])      # [Lq, Lk]
    norm_attn   = softmax(attn, axis=-1)
    h           = norm_attn @ value            # [Lq, Dv]
    returns (h, attn, norm_attn)

Shapes: B=4, Lq=Lk=256, Dq=Dk=Dv=H=512, fp32.

Sharding: 8 cores = (batch b in 0..3) x (Lq half in 0..1); each core owns a
[128, :] slice of Lq for one batch. No collectives.

Per-core layout strategy ("transposed"): keep H on partitions.
  W[h_chunk partitions, (hc, q) free]  = (query @ Wq + bq)^T   (4 chunks of 128)
  U[h_chunk partitions, (hc, k) free]  = (key @ Wk)^T
  For each q: X[:, k] = U + W[:, q] broadcast (DVE/Pool tensor_scalar add,
  per-partition scalar), T = tanh(X) (ScalarE, one big fp32r instruction per
  q-group; ScalarE is the only tanh engine and the kernel's roofline at
  ~109us/core), then attn is accumulated on the TensorEngine: the stationary
  operand is a sliding [128, 64] window over a zero buffer with v at one
  column, so matmul q deposits sum_h v_h * T[h, k] into PSUM row q%64; the
  512 accumulating matmuls assemble attn [q, k] in two half banks in place.
Per 64-row half: softmax along the free dim, then h = norm_attn @ value via
PE transpose, pipelined behind the second half's main loop.  All matmul
inputs use float32r (4x PE throughput, ~1e-4 output error).
"""

import os
import numpy as np

B, LQ, LK, D, H, DV = 4, 256, 256, 512, 512, 512
P = 128  # partitions
NQ = LQ // 2          # q rows per core
NHC = H // P          # 4 h-chunks
NDC = D // P          # 4 d-chunks
NKC = LK // P         # 2 k-chunks
HQ = 64               # q's per attn psum half (M of the matvec matmuls)
# ramp group sizes: small first groups start ScalarE early, small last ones
# shrink the kernel tail
GROUP_SIZES = [1, 1, 2, 4, 4, 4] + [6] * 8 + [7] * 8 + [4, 2, 1, 1]
assert sum(GROUP_SIZES) == NQ
GMAX = max(GROUP_SIZES)

_NC_CACHE = {}


def _build_nc(use_preadd: bool = True):
    import concourse.tile as tile
    from concourse import bacc, mybir
    from concourse.masks import make_identity

    fp32 = mybir.dt.float32
    fp32r = mybir.dt.float32r
    AF = mybir.ActivationFunctionType

    nc = bacc.Bacc("TRN2", target_bir_lowering=False)

    qT_d = nc.dram_tensor("qT", [P, NDC * P], fp32r, kind="ExternalInput")
    kT_d = nc.dram_tensor("keyT", [P, NDC * LK], fp32r, kind="ExternalInput")
    val_d = nc.dram_tensor("val", [P, NKC * DV], fp32r, kind="ExternalInput")
    Wq_d = nc.dram_tensor("Wq", [D, H], fp32r, kind="ExternalInput")
    Wk_d = nc.dram_tensor("Wk", [D, H], fp32r, kind="ExternalInput")
    bqv_d = nc.dram_tensor("bqv", [P, 2 * NHC], fp32, kind="ExternalInput")

    attn_d = nc.dram_tensor("attn_out", [NQ, LK], fp32, kind="ExternalOutput")
    norm_d = nc.dram_tensor("norm_out", [NQ, LK], fp32, kind="ExternalOutput")
    h_d = nc.dram_tensor("h_out", [NQ, DV], fp32, kind="ExternalOutput")

    from contextlib import ExitStack
    with tile.TileContext(nc) as tc:
        with (
            tc.tile_pool(name="statics", bufs=1) as statics,
            tc.tile_pool(name="psum_pre", bufs=1, space="PSUM") as psum_pre,
            tc.tile_pool(name="psum_attn", bufs=1, space="PSUM") as psum_attn,
            tc.tile_pool(name="psum_end", bufs=2, space="PSUM") as psum_end,
            tc.tile_pool(name="endpool", bufs=2) as endpool,
            ExitStack() as ctx_stack,
        ):
            # ---- static loads ----
            tpool = ctx_stack.enter_context(tc.tile_pool(name="tpool", bufs=3))
            xpool = ctx_stack.enter_context(tc.tile_pool(name="xpool", bufs=2))
            QT_all = statics.tile([P, NDC * P], fp32r)
            KT_all = statics.tile([P, NDC * LK], fp32r)
            QT = [QT_all[:, i * P:(i + 1) * P] for i in range(NDC)]
            KT = [KT_all[:, i * LK:(i + 1) * LK] for i in range(NDC)]
            # hc-major packed weights (host pre-packs): tile hc holds the
            # full d-contraction for h-chunk hc -> one DMA unlocks one chunk
            WQ = [statics.tile([P, NDC * P], fp32r, name=f"wq{i}", tag=f"wq{i}") for i in range(NHC)]
            WK = [statics.tile([P, NDC * P], fp32r, name=f"wk{i}", tag=f"wk{i}") for i in range(NHC)]
            VAL_all = statics.tile([P, NKC * DV], fp32r)
            VAL = [VAL_all[:, i * DV:(i + 1) * DV] for i in range(NKC)]
            BQV = statics.tile([P, 2 * NHC], fp32)
            BQ = BQV[:, 0:NHC]
            V4 = BQV[:, NHC:2 * NHC]
            ZW = statics.tile([P, NHC, 2 * HQ - 1], fp32r)
            ident = statics.tile([P, P], fp32)

            # warm the ACT tanh/exp table set before any data arrives
            warm = statics.tile([P, 1], fp32)
            nc.vector.memset(warm, 0.0)
            nc.scalar.activation(warm, warm, AF.Tanh)

            # zwin is zeros except column HQ-1 = v: build it on-device
            # instead of DMAing 260KB of zeros through the startup stream
            nc.vector.memset(ZW.bitcast(mybir.dt.float32), 0.0)
            nc.gpsimd.dma_start(KT_all, kT_d[:, :])
            nc.sync.dma_start(WK[0], Wk_d[0:P, :])
            nc.gpsimd.dma_start(QT_all, qT_d[:, :])
            nc.sync.dma_start(WQ[0], Wq_d[0:P, :])
            nc.sync.dma_start(BQV, bqv_d[:, :])
            nc.vector.tensor_copy(ZW[:, :, HQ - 1], V4)
            for hc in range(1, NHC):
                nc.sync.dma_start(WK[hc], Wk_d[hc * P:(hc + 1) * P, :])
                nc.sync.dma_start(WQ[hc], Wq_d[hc * P:(hc + 1) * P, :])
            nc.gpsimd.dma_start(VAL_all, val_d[:, :])
            make_identity(nc, ident[:, :])

            # ---- precompute W (wq^T + bq) and U (uh^T), H on partitions ----
            Wt = statics.tile([P, NHC * P], fp32)    # [h_in_chunk, (hc, q)]
            Ut = statics.tile([P, NHC * LK], fp32)   # [h_in_chunk, (hc, k)]
            with tc.high_priority():
                for hc in range(NHC):
                    pu = psum_pre.tile([P, LK], fp32, tag="pu", name=f"pu{hc}")
                    for dc in range(NDC):
                        nc.tensor.matmul(
                            pu, WK[hc][:, dc * P:(dc + 1) * P], KT[dc],
                            start=(dc == 0), stop=(dc == NDC - 1),
                        )
                    nc.vector.tensor_copy(Ut[:, hc * LK:(hc + 1) * LK], pu)
                    pw = psum_pre.tile([P, P], fp32, tag="pw", name=f"pw{hc}")
                    for dc in range(NDC):
                        nc.tensor.matmul(
                            pw, WQ[hc][:, dc * P:(dc + 1) * P], QT[dc],
                            start=(dc == 0), stop=(dc == NDC - 1),
                        )
                    nc.vector.tensor_scalar_add(
                        Wt[:, hc * P:(hc + 1) * P], pw, BQ[:, hc:hc + 1]
                    )

            # ---- main loop: tanh cube + attn accumulation (two q-halves) ----
            attn_psh = [
                psum_attn.tile([P, LK], fp32, name=f"aps{j}", tag=f"aps{j}")
                for j in range(NQ // HQ)
            ]
            h_psh = [
                psum_end.tile([P, DV], fp32, name=f"hps{j}", tag=f"hps{j}", bufs=1)
                for j in range(NQ // HQ)
            ]
            attn_sbh = [statics.tile([HQ, LK], fp32, name=f"asb{j}", tag=f"asb{j}")
                        for j in range(NQ // HQ)]
            e_sbh = [statics.tile([HQ, LK], fp32, name=f"esb{j}", tag=f"esb{j}")
                     for j in range(NQ // HQ)]
            norm_sbh = [statics.tile([HQ, LK], fp32, name=f"nsb{j}", tag=f"nsb{j}")
                        for j in range(NQ // HQ)]
            h_sbh = [statics.tile([HQ, DV], fp32, name=f"hsb{j}", tag=f"hsb{j}")
                     for j in range(NQ // HQ)]
            stat = [statics.tile([HQ, 4], fp32, name=f"st{j}", tag=f"st{j}")
                    for j in range(NQ // HQ)]

            def epilogue(j):
                """softmax + h for q rows [HQ*j, HQ*(j+1)) of this core.

                Everything runs at partition base 0; only the output DMAs
                place the rows at their DRAM offsets."""
                rs = slice(HQ * j, HQ * (j + 1))
                asb, esb, nsb, hsb, st = (attn_sbh[j], e_sbh[j], norm_sbh[j],
                                          h_sbh[j], stat[j])
                # No max-subtraction: |attn| <= ||v||_1 ~ 20, exp is fp32-safe
                # and softmax is algebraically identical. exp reads PSUM
                # directly so the h path doesn't wait for the attn copy.
                nc.scalar.activation(esb, attn_psh[j][0:HQ, :], AF.Exp)
                nc.vector.tensor_copy(asb, attn_psh[j][0:HQ, :])
                nc.sync.dma_start(attn_d[rs, :], asb)
                nc.vector.reduce_sum(st[:, 2:3], esb, axis=mybir.AxisListType.X)
                nc.vector.reciprocal(st[:, 3:4], st[:, 2:3])
                nc.vector.tensor_scalar_mul(nsb, esb, st[:, 3:4])
                nc.sync.dma_start(norm_d[rs, :], nsb)
                # h = diag(1/sum) @ (e @ value): transpose e (not norm_attn)
                # so the matmuls don't wait on sum/recip; the normalization
                # rides on the final PSUM->SBUF copy for free.
                for kc in range(NKC):
                    pt = psum_end.tile([P, HQ], fp32, tag="pt")
                    nc.tensor.transpose(
                        pt, esb[:, kc * P:(kc + 1) * P], ident[0:HQ, 0:HQ]
                    )
                    nat = endpool.tile([P, HQ], fp32r, tag="nat")
                    nc.vector.tensor_copy(nat, pt)
                    nc.tensor.matmul(
                        h_psh[j][0:HQ, :], nat, VAL[kc],
                        start=(kc == 0), stop=(kc == NKC - 1),
                    )
                if j == NQ // HQ - 1:
                    # final half: ScalarE is idle at the tail
                    nc.scalar.mul(hsb, h_psh[j][0:HQ, :], st[:, 3:4])
                else:
                    nc.vector.tensor_scalar_mul(hsb, h_psh[j][0:HQ, :],
                                                st[:, 3:4])
                nc.sync.dma_start(h_d[rs, :], hsb)

            q0 = 0
            for g, gsz in enumerate(GROUP_SIZES):
                if g < 2 and gsz == 1:
                    # ramp-in: per-hc ACT-with-bias (no DVE preadd dep), so
                    # tanh for chunk hc starts as soon as Ut/Wt[hc] land
                    T = tpool.tile([P, GMAX * NHC * LK], fp32r, tag="t")
                    q = q0
                    for hc in range(NHC):
                        off = hc * LK
                        nc.scalar.activation(
                            T[:, off:off + LK],
                            Ut[:, hc * LK:(hc + 1) * LK],
                            AF.Tanh,
                            bias=Wt[:, hc * P + q:hc * P + q + 1],
                        )
                elif use_preadd:
                    X = xpool.tile([P, GMAX * NHC * LK], fp32, tag="x")
                    T = tpool.tile([P, GMAX * NHC * LK], fp32r, tag="t")
                    for ql in range(gsz):
                        q = q0 + ql
                        for hc in range(NHC):
                            off = (ql * NHC + hc) * LK
                            eng = nc.gpsimd if hc == NHC - 1 else nc.vector
                            eng.tensor_scalar_add(
                                X[:, off:off + LK],
                                Ut[:, hc * LK:(hc + 1) * LK],
                                Wt[:, hc * P + q:hc * P + q + 1],
                            )
                    fd = gsz * NHC * LK
                    nc.scalar.activation(T[:, :fd], X[:, :fd], AF.Tanh)
                else:
                    T = tpool.tile([P, GMAX * NHC * LK], fp32r, tag="t")
                    for ql in range(gsz):
                        q = q0 + ql
                        for hc in range(NHC):
                            off = (ql * NHC + hc) * LK
                            nc.scalar.activation(
                                T[:, off:off + LK],
                                Ut[:, hc * LK:(hc + 1) * LK],
                                AF.Tanh,
                                bias=Wt[:, hc * P + q:hc * P + q + 1],
                            )
                for ql in range(gsz):
                    q = q0 + ql
                    j, r = q // HQ, q % HQ
                    for hc in range(NHC):
                        off = (ql * NHC + hc) * LK
                        nc.tensor.matmul(
                            attn_psh[j][0:HQ, :],
                            ZW[:, hc, (HQ - 1) - r:(2 * HQ - 1) - r],
                            T[:, off:off + LK],
                            start=(r == 0 and hc == 0),
                            stop=(r == HQ - 1 and hc == NHC - 1),
                        )
                q0 += gsz
                if q0 % HQ == 0:
                    epilogue(q0 // HQ - 1)

    nc.compile()
    return nc


def _get_nc(use_preadd: bool = True):
    key = use_preadd
    if key not in _NC_CACHE:
        _NC_CACHE[key] = _build_nc(use_preadd)
    return _NC_CACHE[key]


def _make_in_maps(query, key, value, Wq, bq, Wk, v):
    v4 = np.ascontiguousarray(v.reshape(NHC, P).T)        # [128, 4]

    def pack_pmaj(a):
        # [n*128, F] -> [128, n*F]: row-chunk i becomes free-dim block i,
        # so one contiguous DMA fills one SBUF tile holding all chunks
        n = a.shape[0] // P
        return np.ascontiguousarray(
            a.reshape(n, P, a.shape[1]).transpose(1, 0, 2).reshape(P, -1)
        )
    bq4 = np.ascontiguousarray(bq.reshape(NHC, P).T)      # [128, 4]

    def hc_major(w):
        # w[dc*128+p, hc*128+h'] -> packed[hc*128+p, dc*128+h']
        return np.ascontiguousarray(
            w.reshape(NDC, P, NHC, P).transpose(2, 1, 0, 3).reshape(D, H)
        ).astype(np.float32)

    Wq = hc_major(np.asarray(Wq, dtype=np.float32))
    Wk = hc_major(np.asarray(Wk, dtype=np.float32))
    in_maps = []
    for core in range(8):
        b, half = core // 2, core % 2
        in_maps.append({
            "qT": pack_pmaj(query[b, half * NQ:(half + 1) * NQ, :].T),
            "keyT": pack_pmaj(key[b].T),
            "val": pack_pmaj(value[b]),
            "Wq": Wq,
            "Wk": Wk,
            "bqv": np.ascontiguousarray(np.concatenate([bq4, v4], axis=1)),
        })
    return in_maps


def kernel(query, key, value, Wq, bq, Wk, v, _trace=False, _use_preadd=True):
    from concourse.bass_utils import run_bass_kernel_spmd

    query = np.asarray(query, dtype=np.float32)
    key = np.asarray(key, dtype=np.float32)
    value = np.asarray(value, dtype=np.float32)
    Wq = np.asarray(Wq, dtype=np.float32)
    bq = np.asarray(bq, dtype=np.float32)
    Wk = np.asarray(Wk, dtype=np.float32)
    v = np.asarray(v, dtype=np.float32)

    nc = _get_nc(_use_preadd)
    in_maps = _make_in_maps(query, key, value, Wq, bq, Wk, v)
    res = run_bass_kernel_spmd(nc, in_maps, core_ids=list(range(8)), trace=_trace)

    h = np.empty((B, LQ, DV), dtype=np.float32)
    attn = np.empty((B, LQ, LK), dtype=np.float32)
    norm = np.empty((B, LQ, LK), dtype=np.float32)
    for core in range(8):
        b, half = core // 2, core % 2
        r = res.results[core]
        sl = slice(half * NQ, (half + 1) * NQ)
        h[b, sl, :] = r["h_out"]
        attn[b, sl, :] = r["attn_out"]
        norm[b, sl, :] = r["norm_out"]
    if _trace:
        kernel._last_results = res
    return (h, attn, norm)


# ---------------------------------------------------------------------------
# CoreSim self-test of a single core's program (no hardware needed).
def _selftest_sim(use_preadd=True):
    from concourse.bass_interp import CoreSim

    rng = np.random.default_rng(0)
    query = rng.standard_normal((B, LQ, D), dtype=np.float32)
    key = rng.standard_normal((B, LK, D), dtype=np.float32)
    value = rng.standard_normal((B, LK, DV), dtype=np.float32)
    Wq = (rng.standard_normal((D, H)) * 0.05).astype(np.float32)
    bq = (rng.standard_normal((H,)) * 0.05).astype(np.float32)
    Wk = (rng.standard_normal((D, H)) * 0.05).astype(np.float32)
    v = (rng.standard_normal((H,)) * 0.05).astype(np.float32)

    nc = _build_nc(use_preadd)
    in_maps = _make_in_maps(query, key, value, Wq, bq, Wk, v)
    core = 3  # b=1, half=1
    sim = CoreSim(nc)
    for name, arr in in_maps[core].items():
        sim.tensor(name)[:] = arr
    if nc.partition_id_tensor is not None:
        sim.tensor(nc.partition_id_tensor.name)[:] = np.array(
            [[core]], dtype=np.uint32)
    sim.simulate(check_with_hw=False)

    # numpy reference for that core
    b, half = core // 2, core % 2
    qs = query[b, half * NQ:(half + 1) * NQ, :]
    wq = qs @ Wq + bq
    uh = key[b] @ Wk
    x = np.tanh(wq[:, None, :] + uh[None, :, :])
    attn_ref = np.einsum("qkh,h->qk", x, v)
    m = attn_ref.max(-1, keepdims=True)
    e = np.exp(attn_ref - m)
    norm_ref = e / e.sum(-1, keepdims=True)
    h_ref = norm_ref @ value[b]

    for name, ref in [("attn_out", attn_ref), ("norm_out", norm_ref),
                      ("h_out", h_ref)]:
        got = np.asarray(sim.tensor(name))
        err = np.abs(got - ref).max()
        rel = err / max(np.abs(ref).max(), 1e-30)
        print(f"{name}: maxabs={err:.3e} rel={rel:.3e}")


if __name__ == "__main__":
    _selftest_sim(use_preadd=os.environ.get("PREADD", "1") == "1")
